# revision 12
# baseline (speedup 1.0000x reference)
"""Trainium2 Bass kernel for nn_CrossAttention (B=2, N=2048, D=768, H=12).

Sharding: (batch, head-group) across 8 cores — core c handles batch c//4 and
heads [3g, 3g+2] where g = c%4. Attention is fully local per (batch, head).

Per-core algorithm (all matmuls in float32r — full-rate PE, ~1e-4 rounding):
  - load x2[b].T (xq_t) and x1[b].T (xkv_t), round f32 -> f32r
  - qT/kT [pd, N] via PE (weights stationary), bias added during PSUM->SBUF
  - v in natural layout [N, pd] (3 heads + a ones column per head -> "v'")
  - per (head, j-chunk, i-half): S^T tile = kT.T-slice @ qT-slice in PSUM,
    exp on ACT -> f32r SBUF, then v'.T @ expP accumulates [pd+1, i] in PSUM;
    row pd is sum_j exp (softmax denominator) via the ones column.
  - divide by denominator (DVE reciprocal + gpsimd partition broadcast),
    DMA out_T [pd, N] per head; host reassembles/reshapes.
"""

import sys

if "/opt/trn_rl_repo" not in sys.path:
    sys.path.insert(0, "/opt/trn_rl_repo")

import numpy as np

import concourse.bass as bass
import concourse.tile as tile
from concourse import bacc, mybir
from concourse.bass_utils import run_bass_kernel_spmd

F32 = mybir.dt.float32
F32R = mybir.dt.float32r
AF = mybir.ActivationFunctionType

B, N, D, H, PD = 2, 2048, 768, 12, 64
HPC = 3  # heads per core
KC = 6  # contraction chunks: 768 / 128
NI = 4  # i (query) chunks of 512
NJ = 16  # j (key) chunks of 128
WV_N = 256  # v-projection rhs width (192 padded to 256 for f32r full rate)

# test harness hooks
TRACE = False
LAST_RESULTS = None

# attention-matmul operand dtype (projections always f32r): F32R or bfloat16
ATT_DT = F32R

_cache: dict = {}


def _emit(tc, xq_t, xkv_t, wq_t, wk_t, wv_t, bq, bk, bv, o_t, loop_iters=1):
    if loop_iters > 1:
        with tc.For_i(0, loop_iters, 1):
            _emit_body(tc, xq_t, xkv_t, wq_t, wk_t, wv_t, bq, bk, bv, o_t)
    else:
        _emit_body(tc, xq_t, xkv_t, wq_t, wk_t, wv_t, bq, bk, bv, o_t)


def _emit_body(tc, xq_t, xkv_t, wq_t, wk_t, wv_t, bq, bk, bv, o_t):
    nc = tc.nc

    import contextlib

    with contextlib.ExitStack() as ctx:
        persist = ctx.enter_context(tc.tile_pool(name="persist", bufs=1))
        stage = ctx.enter_context(tc.tile_pool(name="stage", bufs=2))
        expp = ctx.enter_context(tc.tile_pool(name="expp", bufs=3))
        outp = ctx.enter_context(tc.tile_pool(name="outp", bufs=2))
        smallp = ctx.enter_context(tc.tile_pool(name="smallp", bufs=2))
        ps_pool = ctx.enter_context(tc.tile_pool(name="ps", bufs=2, space="PSUM"))
        po_pool = ctx.enter_context(tc.tile_pool(name="po", bufs=4, space="PSUM"))

        # ---- biases ----
        bq_sb = persist.tile([128, 2], F32)
        bk_sb = persist.tile([128, 2], F32)
        nc.sync.dma_start(bq_sb[:, 0:1], bq[0:128, :])
        nc.sync.dma_start(bq_sb[0:64, 1:2], bq[128:192, :])
        nc.sync.dma_start(bk_sb[:, 0:1], bk[0:128, :])
        nc.sync.dma_start(bk_sb[0:64, 1:2], bk[128:192, :])
        bv_f32 = persist.tile([1, WV_N], F32)
        nc.sync.dma_start(bv_f32[:], bv[:])
        bv_r = persist.tile([1, WV_N], F32R)
        nc.vector.tensor_copy(bv_r[:], bv_f32[:])

        # ones row [1, 128] (f32r) for the v-bias rank-1 matmul
        ones_row_f32 = persist.tile([1, 128], F32)
        nc.vector.memset(ones_row_f32[:], 1.0)
        ones_row_r = persist.tile([1, 128], F32R)
        nc.vector.tensor_copy(ones_row_r[:], ones_row_f32[:])

        # ones [128, 48] (f32) source for v' ones-columns
        ones48 = persist.tile([128, 48], F32)
        nc.vector.memset(ones48[:], 1.0)

        # ---- weights: DMA f32 staging -> round to f32r ----
        def load_weight(wdram, wcols):
            stg = stage.tile([128, 2048], F32, tag="stg")
            w_view = wdram.rearrange("(kc p) w -> p kc w", p=128)  # [128, KC, wcols]
            nc.sync.dma_start(
                stg[:, 0 : KC * wcols].rearrange("p (kc w) -> p kc w", kc=KC), w_view
            )
            w_r = persist.tile([128, KC * wcols], F32R, name=wdram.name + "_r")
            nc.vector.tensor_copy(w_r[:], stg[:, 0 : KC * wcols])
            return w_r

        wq_r = load_weight(wq_t, HPC * PD)
        wk_r = load_weight(wk_t, HPC * PD)
        wv_r = load_weight(wv_t, WV_N)

        # ---- x: DMA f32 chunks -> round to f32r ----
        xq_r = persist.tile([128, KC * N], F32R)
        xkv_r = persist.tile([128, KC * N], F32R)
        for kc in range(KC):
            for src, dst in ((xq_t, xq_r), (xkv_t, xkv_r)):
                stg = stage.tile([128, 2048], F32, tag="stg")
                nc.sync.dma_start(stg[:], src[kc * 128 : (kc + 1) * 128, :])
                nc.vector.tensor_copy(dst[:, kc * N : (kc + 1) * N], stg[:])

        # ---- projections ----
        att_dt = ATT_DT
        qT01 = persist.tile([128, N], att_dt)
        kT01 = persist.tile([128, N], att_dt)
        qT2 = persist.tile([64, N], att_dt)
        kT2 = persist.tile([64, N], att_dt)

        WQK = HPC * PD  # 192
        for w_r, x_r, b_sb, out01, out2 in (
            (wq_r, xq_r, bq_sb, qT01, qT2),
            (wk_r, xkv_r, bk_sb, kT01, kT2),
        ):
            for grp, (m0, msz) in enumerate(((0, 128), (128, 64))):
                out_t = out01 if grp == 0 else out2
                bias_ap = b_sb[0:msz, grp : grp + 1]
                for ic in range(NI):
                    ps = po_pool.tile([msz, 512], F32, tag="po")
                    for kc in range(KC):
                        nc.tensor.matmul(
                            ps[:],
                            w_r[:, kc * WQK + m0 : kc * WQK + m0 + msz],
                            x_r[:, kc * N + ic * 512 : kc * N + (ic + 1) * 512],
                            start=(kc == 0),
                            stop=(kc == KC - 1),
                        )
                    nc.vector.tensor_scalar_add(
                        out_t[:, ic * 512 : (ic + 1) * 512], ps[:], bias_ap
                    )

        # ---- v in natural layout, 3 heads of 64 + ones col -> [128, 48*65] ----
        VW = HPC * (PD + 1)  # 195 per j-tile
        v_sb = persist.tile([128, NJ * VW], att_dt)
        for jt in range(NJ):
            ps = po_pool.tile([128, WV_N], F32, tag="po")
            for kc in range(KC):
                nc.tensor.matmul(
                    ps[:],
                    xkv_r[:, kc * N + jt * 128 : kc * N + (jt + 1) * 128],
                    wv_r[:, kc * WV_N : (kc + 1) * WV_N],
                    start=(kc == 0),
                    stop=False,
                )
            nc.tensor.matmul(ps[:], ones_row_r[:], bv_r[:], start=False, stop=True)
            # copy the 3x64 valid columns into the strided v' layout
            src = ps[:, 0 : HPC * PD].rearrange("p (h c) -> p h c", h=HPC)
            dstv = v_sb[:, jt * VW : (jt + 1) * VW].rearrange(
                "p (h c) -> p h c", h=HPC
            )[:, :, 0:PD]
            nc.vector.tensor_copy(dstv, src)
        # ones columns (column 64 of each head block)
        dst_ones = v_sb[:].rearrange("p (g c) -> p g c", c=PD + 1)[:, :, PD : PD + 1]
        nc.vector.tensor_copy(dst_ones, ones48[:].rearrange("p (g o) -> p g o", o=1))

        # ---- attention ----
        for h in range(HPC):
            qT_h = qT01[h * 64 : (h + 1) * 64, :] if h < 2 else qT2[:]
            kT_h = kT01[h * 64 : (h + 1) * 64, :] if h < 2 else kT2[:]
            po = [
                po_pool.tile([128, 512], F32, tag="po", name=f"po_h{h}_{i}")
                for i in range(NI)
            ]
            for jc in range(NJ):
                vp = v_sb[:, jc * VW + h * (PD + 1) : jc * VW + (h + 1) * (PD + 1)]
                for ih in range(2):
                    pss = ps_pool.tile([128, 1024], F32, tag="ps")
                    for q in range(2):
                        ic = ih * 2 + q
                        nc.tensor.matmul(
                            pss[:, q * 512 : (q + 1) * 512],
                            kT_h[:, jc * 128 : (jc + 1) * 128],
                            qT_h[:, ic * 512 : (ic + 1) * 512],
                            start=True,
                            stop=True,
                        )
                    ex = expp.tile([128, 1024], att_dt, tag="ex")
                    nc.scalar.activation(ex[:], pss[:], AF.Exp)
                    for q in range(2):
                        ic = ih * 2 + q
                        nc.tensor.matmul(
                            po[ic][0 : PD + 1, :],
                            vp,
                            ex[:, q * 512 : (q + 1) * 512],
                            start=(jc == 0),
                            stop=(jc == NJ - 1),
                        )
            for ic in range(NI):
                recip = smallp.tile([1, 512], F32, tag="recip")
                nc.vector.reciprocal(recip[:], po[ic][PD : PD + 1, :])
                bcast = smallp.tile([64, 512], F32, tag="bcast")
                nc.gpsimd.partition_broadcast(bcast[:], recip[:])
                out_sb = outp.tile([64, 512], F32, tag="out")
                nc.vector.tensor_mul(out_sb[:], po[ic][0:PD, :], bcast[:])
                nc.sync.dma_start(o_t[h, :, ic * 512 : (ic + 1) * 512], out_sb[:])


def _build(loop_iters=1):
    key = ("nc", loop_iters, str(ATT_DT))
    if key in _cache:
        return _cache[key]
    nc = bacc.Bacc("TRN2", target_bir_lowering=False, debug=False, num_devices=8)
    xq_t = nc.dram_tensor("xq_t", [D, N], F32, kind="ExternalInput").ap()
    xkv_t = nc.dram_tensor("xkv_t", [D, N], F32, kind="ExternalInput").ap()
    wq_t = nc.dram_tensor("wq_t", [D, HPC * PD], F32, kind="ExternalInput").ap()
    wk_t = nc.dram_tensor("wk_t", [D, HPC * PD], F32, kind="ExternalInput").ap()
    wv_t = nc.dram_tensor("wv_t", [D, WV_N], F32, kind="ExternalInput").ap()
    bq = nc.dram_tensor("bq", [HPC * PD, 1], F32, kind="ExternalInput").ap()
    bk = nc.dram_tensor("bk", [HPC * PD, 1], F32, kind="ExternalInput").ap()
    bv = nc.dram_tensor("bv", [1, WV_N], F32, kind="ExternalInput").ap()
    o_t = nc.dram_tensor("o_t", [HPC, PD, N], F32, kind="ExternalOutput").ap()
    with tile.TileContext(nc) as tc:
        _emit(tc, xq_t, xkv_t, wq_t, wk_t, wv_t, bq, bk, bv, o_t, loop_iters)
    nc.compile()
    _cache[key] = nc
    return nc


def _shard(x1, x2, Wq, bq, Wkv, bkv):
    in_maps = []
    for c in range(8):
        b, g = divmod(c, 4)
        hd = slice(192 * g, 192 * (g + 1))
        wv_pad = np.zeros((D, WV_N), np.float32)
        wv_pad[:, 0 : 3 * PD] = Wkv[D + hd.start : D + hd.stop].T
        bv_pad = np.zeros((1, WV_N), np.float32)
        bv_pad[0, 0 : 3 * PD] = bkv[D + hd.start : D + hd.stop]
        in_maps.append(
            {
                "xq_t": np.ascontiguousarray(x2[b].T),
                "xkv_t": np.ascontiguousarray(x1[b].T),
                "wq_t": np.ascontiguousarray(Wq[hd].T),
                "wk_t": np.ascontiguousarray(Wkv[hd].T),
                "wv_t": wv_pad,
                "bq": np.ascontiguousarray(bq[hd].reshape(-1, 1)),
                "bk": np.ascontiguousarray(bkv[hd].reshape(-1, 1)),
                "bv": bv_pad,
            }
        )
    return in_maps


def kernel(x1, x2, Wq, bq, Wkv, bkv):
    global LAST_RESULTS
    x1 = np.asarray(x1, dtype=np.float32)
    x2 = np.asarray(x2, dtype=np.float32)
    Wq = np.asarray(Wq, dtype=np.float32)
    bq = np.asarray(bq, dtype=np.float32)
    Wkv = np.asarray(Wkv, dtype=np.float32)
    bkv = np.asarray(bkv, dtype=np.float32)

    nc = _build()
    in_maps = _shard(x1, x2, Wq, bq, Wkv, bkv)
    res = run_bass_kernel_spmd(
        nc, in_maps, core_ids=list(range(8)), trace=TRACE
    )
    LAST_RESULTS = res

    out = np.empty((B, H, N, PD), np.float32)
    for c in range(8):
        b, g = divmod(c, 4)
        ot = res.results[c]["o_t"]  # (3, 64, 2048)
        out[b, 3 * g : 3 * g + 3] = ot.transpose(0, 2, 1)
    return out.reshape(B, N, D)


# revision 14
# speedup vs baseline: 1.0887x; 1.0887x over previous
"""Trainium2 Bass kernel for nn_CrossAttention (B=2, N=2048, D=768, H=12).

Sharding: (batch, head-group) across 8 cores — core c handles batch c//4 and
heads [3g, 3g+2] where g = c%4. Attention is fully local per (batch, head).

Per-core algorithm (all matmuls in float32r — full-rate PE, ~1e-4 rounding):
  - load x2[b].T (xq_t) and x1[b].T (xkv_t), round f32 -> f32r
  - qT/kT [pd, N] via PE (weights stationary), bias added during PSUM->SBUF
  - v in natural layout [N, pd] (3 heads + a ones column per head -> "v'")
  - per (head, j-chunk, i-half): S^T tile = kT.T-slice @ qT-slice in PSUM,
    exp on ACT -> f32r SBUF, then v'.T @ expP accumulates [pd+1, i] in PSUM;
    row pd is sum_j exp (softmax denominator) via the ones column.
  - divide by denominator (DVE reciprocal + gpsimd partition broadcast),
    DMA out_T [pd, N] per head; host reassembles/reshapes.
"""

import sys

if "/opt/trn_rl_repo" not in sys.path:
    sys.path.insert(0, "/opt/trn_rl_repo")

import numpy as np

import concourse.bass as bass
import concourse.tile as tile
from concourse import bacc, mybir
from concourse.bass_utils import run_bass_kernel_spmd

F32 = mybir.dt.float32
F32R = mybir.dt.float32r
AF = mybir.ActivationFunctionType

B, N, D, H, PD = 2, 2048, 768, 12, 64
HPC = 3  # heads per core
KC = 6  # contraction chunks: 768 / 128
NI = 4  # i (query) chunks of 512
NJ = 16  # j (key) chunks of 128
WV_N = 256  # v-projection rhs width (192 padded to 256 for f32r full rate)

# test harness hooks
TRACE = False
LAST_RESULTS = None

# attention-matmul operand dtype (projections always f32r): F32R or bfloat16
ATT_DT = F32R
# emit head-0/1 projections + v first so attention starts earlier (~22us win)
REORDER = True

_cache: dict = {}


def _emit(tc, xq_t, xkv_t, wq_t, wk_t, wv_t, bq, bk, bv, o_t, loop_iters=1):
    if loop_iters > 1:
        with tc.For_i(0, loop_iters, 1):
            _emit_body(tc, xq_t, xkv_t, wq_t, wk_t, wv_t, bq, bk, bv, o_t)
    else:
        _emit_body(tc, xq_t, xkv_t, wq_t, wk_t, wv_t, bq, bk, bv, o_t)


def _emit_body(tc, xq_t, xkv_t, wq_t, wk_t, wv_t, bq, bk, bv, o_t):
    nc = tc.nc

    import contextlib

    with contextlib.ExitStack() as ctx:
        persist = ctx.enter_context(tc.tile_pool(name="persist", bufs=1))
        stage = ctx.enter_context(tc.tile_pool(name="stage", bufs=2))
        expp = ctx.enter_context(tc.tile_pool(name="expp", bufs=3))
        outp = ctx.enter_context(tc.tile_pool(name="outp", bufs=2))
        smallp = ctx.enter_context(tc.tile_pool(name="smallp", bufs=2))
        ps_pool = ctx.enter_context(tc.tile_pool(name="ps", bufs=2, space="PSUM"))
        po_pool = ctx.enter_context(tc.tile_pool(name="po", bufs=4, space="PSUM"))

        # ---- biases ----
        bq_sb = persist.tile([128, 2], F32)
        bk_sb = persist.tile([128, 2], F32)
        nc.sync.dma_start(bq_sb[:, 0:1], bq[0:128, :])
        nc.sync.dma_start(bq_sb[0:64, 1:2], bq[128:192, :])
        nc.sync.dma_start(bk_sb[:, 0:1], bk[0:128, :])
        nc.sync.dma_start(bk_sb[0:64, 1:2], bk[128:192, :])
        bv_f32 = persist.tile([1, WV_N], F32)
        nc.sync.dma_start(bv_f32[:], bv[:])
        bv_r = persist.tile([1, WV_N], F32R)
        nc.vector.tensor_copy(bv_r[:], bv_f32[:])

        # ones row [1, 128] (f32r) for the v-bias rank-1 matmul
        ones_row_f32 = persist.tile([1, 128], F32)
        nc.vector.memset(ones_row_f32[:], 1.0)
        ones_row_r = persist.tile([1, 128], F32R)
        nc.vector.tensor_copy(ones_row_r[:], ones_row_f32[:])

        # ones [128, 48] (f32) source for v' ones-columns
        ones48 = persist.tile([128, 48], F32)
        nc.vector.memset(ones48[:], 1.0)

        # ---- weights: DMA f32 staging -> round to f32r ----
        def load_weight(wdram, wcols):
            stg = stage.tile([128, 2048], F32, tag="stg")
            w_view = wdram.rearrange("(kc p) w -> p kc w", p=128)  # [128, KC, wcols]
            nc.sync.dma_start(
                stg[:, 0 : KC * wcols].rearrange("p (kc w) -> p kc w", kc=KC), w_view
            )
            w_r = persist.tile([128, KC * wcols], F32R, name=wdram.name + "_r")
            nc.vector.tensor_copy(w_r[:], stg[:, 0 : KC * wcols])
            return w_r

        wq_r = load_weight(wq_t, HPC * PD)
        wk_r = load_weight(wk_t, HPC * PD)
        wv_r = load_weight(wv_t, WV_N)

        # ---- x: DMA f32 chunks -> round to f32r ----
        xq_r = persist.tile([128, KC * N], F32R)
        xkv_r = persist.tile([128, KC * N], F32R)
        for kc in range(KC):
            for src, dst in ((xq_t, xq_r), (xkv_t, xkv_r)):
                stg = stage.tile([128, 2048], F32, tag="stg")
                nc.sync.dma_start(stg[:], src[kc * 128 : (kc + 1) * 128, :])
                nc.vector.tensor_copy(dst[:, kc * N : (kc + 1) * N], stg[:])

        # ---- projections ----
        att_dt = ATT_DT
        qT01 = persist.tile([128, N], att_dt)
        kT01 = persist.tile([128, N], att_dt)
        qT2 = persist.tile([64, N], att_dt)
        kT2 = persist.tile([64, N], att_dt)

        WQK = HPC * PD  # 192
        def proj_group(w_r, x_r, b_sb, out_t, grp, m0, msz):
            bias_ap = b_sb[0:msz, grp : grp + 1]
            for ic in range(NI):
                ps = po_pool.tile([msz, 512], F32, tag="po", name=f"pj{grp}_{ic}")
                for kc in range(KC):
                    nc.tensor.matmul(
                        ps[:],
                        w_r[:, kc * WQK + m0 : kc * WQK + m0 + msz],
                        x_r[:, kc * N + ic * 512 : kc * N + (ic + 1) * 512],
                        start=(kc == 0),
                        stop=(kc == KC - 1),
                    )
                nc.vector.tensor_scalar_add(
                    out_t[:, ic * 512 : (ic + 1) * 512], ps[:], bias_ap
                )

        if not REORDER:
            proj_group(wq_r, xq_r, bq_sb, qT01, 0, 0, 128)
            proj_group(wq_r, xq_r, bq_sb, qT2, 1, 128, 64)
            proj_group(wk_r, xkv_r, bk_sb, kT01, 0, 0, 128)
            proj_group(wk_r, xkv_r, bk_sb, kT2, 1, 128, 64)

        if REORDER:
            proj_group(wq_r, xq_r, bq_sb, qT01, 0, 0, 128)
            proj_group(wk_r, xkv_r, bk_sb, kT01, 0, 0, 128)

        # ---- v in natural layout, 3 heads of 64 + ones col -> [128, 48*65] ----
        VW = HPC * (PD + 1)  # 195 per j-tile
        v_sb = persist.tile([128, NJ * VW], att_dt)
        for jt in range(NJ):
            ps = po_pool.tile([128, WV_N], F32, tag="po")
            for kc in range(KC):
                nc.tensor.matmul(
                    ps[:],
                    xkv_r[:, kc * N + jt * 128 : kc * N + (jt + 1) * 128],
                    wv_r[:, kc * WV_N : (kc + 1) * WV_N],
                    start=(kc == 0),
                    stop=False,
                )
            nc.tensor.matmul(ps[:], ones_row_r[:], bv_r[:], start=False, stop=True)
            # copy the 3x64 valid columns into the strided v' layout
            src = ps[:, 0 : HPC * PD].rearrange("p (h c) -> p h c", h=HPC)
            dstv = v_sb[:, jt * VW : (jt + 1) * VW].rearrange(
                "p (h c) -> p h c", h=HPC
            )[:, :, 0:PD]
            nc.vector.tensor_copy(dstv, src)
        # ones columns (column 64 of each head block)
        dst_ones = v_sb[:].rearrange("p (g c) -> p g c", c=PD + 1)[:, :, PD : PD + 1]
        nc.vector.tensor_copy(dst_ones, ones48[:].rearrange("p (g o) -> p g o", o=1))

        if REORDER:
            proj_group(wq_r, xq_r, bq_sb, qT2, 1, 128, 64)
            proj_group(wk_r, xkv_r, bk_sb, kT2, 1, 128, 64)

        # ---- attention ----
        for h in range(HPC):
            qT_h = qT01[h * 64 : (h + 1) * 64, :] if h < 2 else qT2[:]
            kT_h = kT01[h * 64 : (h + 1) * 64, :] if h < 2 else kT2[:]
            po = [
                po_pool.tile([128, 512], F32, tag="po", name=f"po_h{h}_{i}")
                for i in range(NI)
            ]
            for jc in range(NJ):
                vp = v_sb[:, jc * VW + h * (PD + 1) : jc * VW + (h + 1) * (PD + 1)]
                for ih in range(2):
                    pss = ps_pool.tile([128, 1024], F32, tag="ps")
                    for q in range(2):
                        ic = ih * 2 + q
                        nc.tensor.matmul(
                            pss[:, q * 512 : (q + 1) * 512],
                            kT_h[:, jc * 128 : (jc + 1) * 128],
                            qT_h[:, ic * 512 : (ic + 1) * 512],
                            start=True,
                            stop=True,
                        )
                    ex = expp.tile([128, 1024], att_dt, tag="ex")
                    nc.scalar.activation(ex[:], pss[:], AF.Exp)
                    for q in range(2):
                        ic = ih * 2 + q
                        nc.tensor.matmul(
                            po[ic][0 : PD + 1, :],
                            vp,
                            ex[:, q * 512 : (q + 1) * 512],
                            start=(jc == 0),
                            stop=(jc == NJ - 1),
                        )
            for ic in range(NI):
                recip = smallp.tile([1, 512], F32, tag="recip")
                nc.vector.reciprocal(recip[:], po[ic][PD : PD + 1, :])
                bcast = smallp.tile([64, 512], F32, tag="bcast")
                nc.gpsimd.partition_broadcast(bcast[:], recip[:])
                out_sb = outp.tile([64, 512], F32, tag="out")
                nc.vector.tensor_mul(out_sb[:], po[ic][0:PD, :], bcast[:])
                nc.sync.dma_start(o_t[h, :, ic * 512 : (ic + 1) * 512], out_sb[:])


def _build(loop_iters=1):
    key = ("nc", loop_iters, str(ATT_DT), REORDER)
    if key in _cache:
        return _cache[key]
    nc = bacc.Bacc("TRN2", target_bir_lowering=False, debug=False, num_devices=8)
    xq_t = nc.dram_tensor("xq_t", [D, N], F32, kind="ExternalInput").ap()
    xkv_t = nc.dram_tensor("xkv_t", [D, N], F32, kind="ExternalInput").ap()
    wq_t = nc.dram_tensor("wq_t", [D, HPC * PD], F32, kind="ExternalInput").ap()
    wk_t = nc.dram_tensor("wk_t", [D, HPC * PD], F32, kind="ExternalInput").ap()
    wv_t = nc.dram_tensor("wv_t", [D, WV_N], F32, kind="ExternalInput").ap()
    bq = nc.dram_tensor("bq", [HPC * PD, 1], F32, kind="ExternalInput").ap()
    bk = nc.dram_tensor("bk", [HPC * PD, 1], F32, kind="ExternalInput").ap()
    bv = nc.dram_tensor("bv", [1, WV_N], F32, kind="ExternalInput").ap()
    o_t = nc.dram_tensor("o_t", [HPC, PD, N], F32, kind="ExternalOutput").ap()
    with tile.TileContext(nc) as tc:
        _emit(tc, xq_t, xkv_t, wq_t, wk_t, wv_t, bq, bk, bv, o_t, loop_iters)
    nc.compile()
    _cache[key] = nc
    return nc


def _shard(x1, x2, Wq, bq, Wkv, bkv):
    in_maps = []
    for c in range(8):
        b, g = divmod(c, 4)
        hd = slice(192 * g, 192 * (g + 1))
        wv_pad = np.zeros((D, WV_N), np.float32)
        wv_pad[:, 0 : 3 * PD] = Wkv[D + hd.start : D + hd.stop].T
        bv_pad = np.zeros((1, WV_N), np.float32)
        bv_pad[0, 0 : 3 * PD] = bkv[D + hd.start : D + hd.stop]
        in_maps.append(
            {
                "xq_t": np.ascontiguousarray(x2[b].T),
                "xkv_t": np.ascontiguousarray(x1[b].T),
                "wq_t": np.ascontiguousarray(Wq[hd].T),
                "wk_t": np.ascontiguousarray(Wkv[hd].T),
                "wv_t": wv_pad,
                "bq": np.ascontiguousarray(bq[hd].reshape(-1, 1)),
                "bk": np.ascontiguousarray(bkv[hd].reshape(-1, 1)),
                "bv": bv_pad,
            }
        )
    return in_maps


def kernel(x1, x2, Wq, bq, Wkv, bkv):
    global LAST_RESULTS
    x1 = np.asarray(x1, dtype=np.float32)
    x2 = np.asarray(x2, dtype=np.float32)
    Wq = np.asarray(Wq, dtype=np.float32)
    bq = np.asarray(bq, dtype=np.float32)
    Wkv = np.asarray(Wkv, dtype=np.float32)
    bkv = np.asarray(bkv, dtype=np.float32)

    nc = _build()
    in_maps = _shard(x1, x2, Wq, bq, Wkv, bkv)
    res = run_bass_kernel_spmd(
        nc, in_maps, core_ids=list(range(8)), trace=TRACE
    )
    LAST_RESULTS = res

    out = np.empty((B, H, N, PD), np.float32)
    for c in range(8):
        b, g = divmod(c, 4)
        ot = res.results[c]["o_t"]  # (3, 64, 2048)
        out[b, 3 * g : 3 * g + 3] = ot.transpose(0, 2, 1)
    return out.reshape(B, N, D)
